# revision 63
# baseline (speedup 1.0000x reference)
"""MultiHeadAttention (RoPE + QK-RMSNorm, non-causal) on 8 trn2 NeuronCores.

Sharding: batch (2) x head-groups (4 heads each) -> 8 cores. All-f16 dataflow
(inputs converted host-side; fp32 PSUM accumulation). Per core:
  - streamed QKV passes (weight-stationary, cc-accumulated): E0 O0 | E1 | O1 | V0
    pre-attention, V1 interleaved into attention as filler work
  - RMS inverse via scalar Sqrt + vector fast reciprocal (no act-table thrash)
  - RoPE on vector in f16; gamma*invrms broadcast via small PE matmuls
  - attention in (pair, 512-query-block) blocks: per kv-chunk j one
    [128,1024] exp on the scalar engine (the pace-setter), 2 score MMs +
    2 AV MMs (N=512, f16); denominator via ones-column of V
  - pair-1 prep, V1 projection+transpose and the first output-projection
    chunks run as fillers inside the attention instruction streams
Host: sums the 4 partials per batch (f16 -> f32) and adds b_out.
"""
import math
import numpy as np

import concourse.bass as bass
from concourse import bacc
import concourse.mybir as mybir
import concourse.tile as tile
from concourse.bass_utils import run_bass_kernel_spmd
from concourse.masks import make_identity

F32 = mybir.dt.float32
F16 = mybir.dt.float16
AF = mybir.ActivationFunctionType

B, L, C, H, D = 2, 2048, 1024, 16, 64
NCORES = 8
ROPE_THETA = 10000.0
RMS_EPS = 1e-6
NPAIR = 2
LQB = 512        # query block size in attention
NKV = L // 128   # 16 kv chunks


def _build_program(dbg=False):
    nc = bacc.Bacc("TRN2", target_bir_lowering=False, debug=False)

    xt_d = nc.dram_tensor("xt", [C, L], F16, kind="ExternalInput")
    wq_d = nc.dram_tensor("wq", [C, 768], F16, kind="ExternalInput")
    bias_d = nc.dram_tensor("bias6", [6, 128], F32, kind="ExternalInput")
    cos_d = nc.dram_tensor("cost", [128, L], F16, kind="ExternalInput")
    sin_d = nc.dram_tensor("sint", [128, L], F16, kind="ExternalInput")
    ind_d = nc.dram_tensor("ind", [128, 4], F16, kind="ExternalInput")
    gind_d = nc.dram_tensor("gind", [4, 4 * 128], F16, kind="ExternalInput")
    wo_d = nc.dram_tensor("wo", [256, 1024], F16, kind="ExternalInput")
    out_d = nc.dram_tensor("out", [L, C], F16, kind="ExternalOutput")

    with tile.TileContext(nc) as tc:
        with tc.tile_pool(name="const", bufs=1) as cp:
            # ---- input DMAs: wq/xt stream first (pass1 is gated on them) ----
            # spread input loads across engine DMA queues for parallel HBM
            # streams (each engine trigger feeds its own queue)
            xw = tc.alloc_tile_pool(name="xw", bufs=1)
            dmae = [nc.sync, nc.scalar, nc.gpsimd]
            xt_sb, wq_sb = [], []
            for cc in range(8):
                wqi = xw.tile([128, 768], F16, tag=f"w{cc}", name=f"w{cc}")
                dmae[cc % 3].dma_start(out=wqi,
                                       in_=wq_d[cc * 128:(cc + 1) * 128, :])
                wq_sb.append(wqi)
                xti = xw.tile([128, L], F16, tag=f"x{cc}", name=f"x{cc}")
                dmae[(cc + 1) % 3].dma_start(
                    out=xti, in_=xt_d[cc * 128:(cc + 1) * 128, :])
                xt_sb.append(xti)
            bias_t = cp.tile([128, 6], F32, tag="bias")
            nc.sync.dma_start(out=bias_t, in_=bias_d[:, :].transpose([1, 0]))
            cos_t = cp.tile([128, L], F16, tag="cos")
            nc.sync.dma_start(out=cos_t, in_=cos_d[:, :])
            sin_t = cp.tile([128, L], F16, tag="sin")
            nc.sync.dma_start(out=sin_t, in_=sin_d[:, :])
            ind_t = cp.tile([128, 4], F16, tag="ind")
            nc.sync.dma_start(out=ind_t, in_=ind_d[:, :])
            gind_t = cp.tile([4, 4 * 128], F16, tag="gind")
            nc.sync.dma_start(out=gind_t, in_=gind_d[:, :])
            wo_t = [cp.tile([128, 1024], F16, tag=f"wo{p}", name=f"wo{p}")
                    for p in range(2)]
            for p in range(2):
                nc.sync.dma_start(out=wo_t[p],
                                  in_=wo_d[p * 128:(p + 1) * 128, :])
            eps_t = cp.tile([4, 1], F32, tag="eps")
            nc.vector.memset(eps_t[:, :], RMS_EPS)
            # pre-warm the Identity and Sqrt act-table sets while everything
            # waits on input DMA — their first real uses sit on the
            # pre-attention critical cascade (bias-adds gate the QKV passes)
            warm_t = cp.tile([4, 1], F32, tag="warm")
            nc.scalar.add(warm_t[:, :], eps_t[:, :], 0.0)
            nc.scalar.activation(warm_t[:, :], eps_t[:, :], AF.Sqrt)
            ones_t = cp.tile([128, 64], F16, tag="ones")
            nc.vector.memset(ones_t[:, :], 1.0)
            onecol = cp.tile([128, 2], F16, tag="onecol")
            nc.vector.memset(onecol[:, :], 1.0)
            ident = cp.tile([128, 128], F16, tag="ident")
            make_identity(nc, ident[:, :])

            # ---- long-lived attention operands ----
            lv = tc.alloc_tile_pool(name="live", bufs=1)
            qT, kT, vseq = [], [], []
            for p in range(NPAIR):
                qT.append(lv.tile([128, L], F16, tag=f"qT{p}", name=f"qT{p}"))
                kT.append(lv.tile([128, L], F16, tag=f"kT{p}", name=f"kT{p}"))
                vseq.append([lv.tile([128, 130], F16, tag=f"vs{p}_{lw}",
                                     name=f"vs{p}_{lw}") for lw in range(NKV)])
            oT = [lv.tile([128, L], F16, tag=f"oT{p}", name=f"oT{p}")
                  for p in range(NPAIR)]

            chk = tc.alloc_tile_pool(name="chunks", bufs=1)
            # oc order in wq columns: E0 O0 V0 E1 O1 V1
            chunks = [chk.tile([128, L], F16, tag=f"c{i}", name=f"c{i}")
                      for i in range(6)]
            E0, O0, V0c, E1, O1, V1c = (chunks[0], chunks[1], chunks[2],
                                        chunks[3], chunks[4], chunks[5])

            scr = tc.alloc_tile_pool(name="scratch", bufs=1)
            # rope temporaries / squares (rotating tags)
            # rms/invf fp32, invrs f16 per pair (tags rotate p0 -> p1)
            dn = tc.alloc_tile_pool(name="dn", bufs=2)

            def qkv_pass_mms(ps_tiles, oc, cc_range, lqs=(0, 1, 2, 3)):
                for cc in cc_range:
                    for lq in lqs:
                        nc.tensor.matmul(
                            ps_tiles[lq][:, :],
                            wq_sb[cc][:, oc * 128:(oc + 1) * 128],
                            xt_sb[cc][:, lq * 512:(lq + 1) * 512],
                            start=(cc == 0), stop=(cc == 7))

            def bias_add(ps_tiles, oc, lq):
                # scalar engine (idle pre-attention); vector is the critical
                # pre-attention chain
                nc.scalar.add(
                    chunks[oc][:, lq * 512:(lq + 1) * 512],
                    ps_tiles[lq][:, :], bias_t[:, oc:oc + 1])

            # ---- pass 1: E0 + O0 (8 psum banks, dma-gated) ----
            q1 = tc.alloc_tile_pool(name="q1", bufs=1, space="PSUM")
            ps1 = {(oc, lq): q1.tile([128, 512], F32, tag=f"p{oc}_{lq}",
                                     name=f"p{oc}_{lq}")
                   for oc in (0, 1) for lq in range(4)}
            for cc in range(8):
                for oc in (0, 1):
                    for lq in range(4):
                        nc.tensor.matmul(
                            ps1[(oc, lq)][:, :],
                            wq_sb[cc][:, oc * 128:(oc + 1) * 128],
                            xt_sb[cc][:, lq * 512:(lq + 1) * 512],
                            start=(cc == 0), stop=(cc == 7))
            for oc in (0, 1):
                for lq in range(4):
                    nc.vector.tensor_scalar_add(
                        chunks[oc][:, lq * 512:(lq + 1) * 512],
                        ps1[(oc, lq)][:, :], bias_t[:, oc:oc + 1])
            q1.release()

            # ---- passes 2-4 on 8 independent psum tags (a0-3: E1 then V0;
            # b0-3: ps4_0 slices, then O1, then gind-M chunks) so no pass
            # waits on the previous pass's scalar bias-add ----
            q2 = tc.alloc_tile_pool(name="q2", bufs=1, space="PSUM")

            # vector: squares of pair0 (reads E0/O0 after pass-1 bias adds)
            sqE = scr.tile([128, L], F16, tag="tC", name="sqE0")
            nc.vector.tensor_mul(sqE[:, :], E0[:, :], E0[:, :])
            sqO = scr.tile([128, L], F16, tag="tD", name="sqO0")
            nc.vector.tensor_mul(sqO[:, :], O0[:, :], O0[:, :])

            # PE: pass2 (E1) on a-tags
            t2 = [q2.tile([128, 512], F32, tag=f"a{lq}", name=f"e1_{lq}")
                  for lq in range(4)]
            qkv_pass_mms(t2, 3, range(8))
            for lq in range(4):
                bias_add(t2, 3, lq)

            # ps4_0 as 4 slices on b-tags, per-slice Sqrt0 -> rms0
            rms0 = scr.tile([4, L], F32, tag="rms", name="rms0")
            for sl in range(4):
                tb = q2.tile([128, 512], F32, tag=f"b{sl}", name=f"p40_{sl}")
                nc.tensor.matmul(tb[0:4, :], ind_t[:, :],
                                 sqE[:, sl * 512:(sl + 1) * 512],
                                 start=True, stop=False)
                nc.tensor.matmul(tb[0:4, :], ind_t[:, :],
                                 sqO[:, sl * 512:(sl + 1) * 512],
                                 start=False, stop=True)
                nc.scalar.activation(rms0[:, sl * 512:(sl + 1) * 512],
                                     tb[0:4, :], AF.Sqrt,
                                     scale=1.0 / 64.0, bias=eps_t[:, 0:1])

            # vector: rope pair0 (f16)
            tC = scr.tile([128, L], F16, tag="tC", name="tC0")
            nc.vector.tensor_mul(tC[:, :], E0[:, :], cos_t[:, :])
            tD = scr.tile([128, L], F16, tag="tD", name="tD0")
            nc.vector.tensor_mul(tD[:, :], O0[:, :], sin_t[:, :])
            rA = scr.tile([128, L], F16, tag="rA", name="rA0")
            nc.vector.tensor_sub(rA[:, :], tC[:, :], tD[:, :])
            tC2 = scr.tile([128, L], F16, tag="tC", name="tC0b")
            nc.vector.tensor_mul(tC2[:, :], E0[:, :], sin_t[:, :])
            tD2 = scr.tile([128, L], F16, tag="tD", name="tD0b")
            nc.vector.tensor_mul(tD2[:, :], O0[:, :], cos_t[:, :])
            rB = scr.tile([128, L], F16, tag="rB", name="rB0")
            nc.vector.tensor_add(rB[:, :], tC2[:, :], tD2[:, :])

            # vector: invrs0 = recip(rms0) -> f16
            invf0 = scr.tile([4, L], F32, tag="invf", name="invf0")
            nc.vector.reciprocal_approx_fast(invf0[:, :], rms0[:, :])
            invrs0 = scr.tile([4, L], F16, tag="invrs", name="invrs0")
            nc.vector.tensor_copy(invrs0[:, :], invf0[:, :])

            # PE: pass3 (O1) on b-tags (rotation waits only the tiny
            # Sqrt0 slice reads, not a bias cascade)
            t3 = [q2.tile([128, 512], F32, tag=f"b{lq}", name=f"o1_{lq}")
                  for lq in range(4)]
            qkv_pass_mms(t3, 4, range(8))
            for lq in range(4):
                bias_add(t3, 4, lq)
            # pair-1 rms chain runs as attention fillers; rms1 declared here
            # (tag rotates after recip0 read rms0)
            rms1 = scr.tile([4, L], F32, tag="rms", name="rms1")

            # gamma*invrms broadcast + apply for pair0 (M chunks on b-tags)
            # interleaved with pass4 (V0, a-tags) so the PE never idles on
            # the vector M-apply muls
            sE = scr.tile([128, L], F16, tag="sE", name="sE0")
            sO = scr.tile([128, L], F16, tag="sO", name="sO0")
            t4 = [q2.tile([128, 512], F32, tag=f"a{lq}", name=f"v0_{lq}")
                  for lq in range(4)]
            k = 0
            for kind, (rt, st) in enumerate(((rA, sE), (rB, sO))):
                gsl = gind_t[:, kind * 128:(kind + 1) * 128]
                for sl in range(4):
                    mm = q2.tile([128, 512], F32, tag=f"b{k % 4}",
                                 name=f"m0_{k}")
                    nc.tensor.matmul(mm[:, :], gsl,
                                     invrs0[:, sl * 512:(sl + 1) * 512],
                                     start=True, stop=True)
                    nc.vector.tensor_mul(
                        st[:, sl * 512:(sl + 1) * 512],
                        rt[:, sl * 512:(sl + 1) * 512], mm[:, :])
                    qkv_pass_mms(t4, 2, range(k, k + 1))
                    k += 1
            for lq in range(4):
                bias_add(t4, 2, lq)

            def reloc(p, sEt, sOt, engs=dmae):
                # spread across engine DMA queues: the 8 copies gate the
                # first score matmuls. Mid-attention callers must exclude
                # the scalar engine: a trigger waiting on the pair-1 apply
                # would block the whole exp stream behind it.
                n = [0]

                def _d(out, in_):
                    engs[n[0] % len(engs)].dma_start(out=out, in_=in_)
                    n[0] += 1
                for blk in range(2):
                    _d(qT[p][blk * 64:blk * 64 + 32, :],
                       sEt[blk * 32:(blk + 1) * 32, :])
                    _d(qT[p][blk * 64 + 32:blk * 64 + 64, :],
                       sOt[blk * 32:(blk + 1) * 32, :])
                    _d(kT[p][blk * 64:blk * 64 + 32, :],
                       sEt[64 + blk * 32:64 + (blk + 1) * 32, :])
                    _d(kT[p][blk * 64 + 32:blk * 64 + 64, :],
                       sOt[64 + blk * 32:64 + (blk + 1) * 32, :])

            reloc(0, sE, sO)

            q2.release()

            # ================= attention phase =================
            # pool default bufs=2 (tags "s", "aux"); oA/oB override to 1.
            # PSUM budget: s 2x4KB + oA 2KB + oB 2KB + aux 2x2KB = 16KB = 8 banks
            ap = tc.alloc_tile_pool(name="att", bufs=2, space="PSUM")
            # deep e rotation: lets the exp stream run ahead while early AVs
            # wait for the V0 transpose fillers to land
            ep = tc.alloc_tile_pool(name="exp", bufs=8)
            ov = tc.alloc_tile_pool(name="ov", bufs=2)

            def v_transpose(p, Vc, lw):
                pt = ap.tile([128, 128], F16, tag="aux", name=f"pt{p}_{lw}")
                nc.tensor.transpose(pt[:, :], Vc[:, lw * 128:(lw + 1) * 128],
                                    ident[:, :])
                vv = vseq[p][lw].rearrange("a (h x) -> a h x", h=2)
                nc.vector.tensor_copy(
                    vv[:, :, 0:64],
                    pt[:, :].rearrange("a (h x) -> a h x", h=2))
                nc.vector.tensor_copy(vv[:, :, 64], onecol[:, :])

            # ---- filler closures, popped inside the attention p0 loop ----
            fill = []

            # V0 transposes as the FIRST fillers: vtr_j pops at iteration j,
            # one ahead of AV_j (emitted at iteration j+1), so they interleave
            # with the exp stream instead of gating attention start
            for lw in range(NKV):
                fill.append(lambda lw=lw: v_transpose(0, V0c, lw))

            # pair-1 squares + ps4_1 + Sqrt1 as fillers (frees the q2 pool
            # release from the Sqrt1 chain). Grouped in one closure so the
            # act-table switches (Sqrt in, Exp back) happen exactly once.
            sq1t = {}

            def _sq1():
                s1 = scr.tile([128, L], F16, tag="tC", name="sqE1")
                nc.vector.tensor_mul(s1[:, :], E1[:, :], E1[:, :])
                sq1t["E"] = s1

            def _sq1b():
                s1 = scr.tile([128, L], F16, tag="tD", name="sqO1")
                nc.vector.tensor_mul(s1[:, :], O1[:, :], O1[:, :])
                sq1t["O"] = s1
            fill.append(_sq1)
            fill.append(_sq1b)

            def _ps4_1():
                for sl in range(4):
                    t = ap.tile([128, 512], F32, tag="aux", name=f"ps4s{sl}")
                    nc.tensor.matmul(t[0:4, :], ind_t[:, :],
                                     sq1t["E"][:, sl * 512:(sl + 1) * 512],
                                     start=True, stop=False)
                    nc.tensor.matmul(t[0:4, :], ind_t[:, :],
                                     sq1t["O"][:, sl * 512:(sl + 1) * 512],
                                     start=False, stop=True)
                    nc.scalar.activation(rms1[:, sl * 512:(sl + 1) * 512],
                                         t[0:4, :], AF.Sqrt,
                                         scale=1.0 / 64.0, bias=eps_t[:, 0:1])
            fill.append(_ps4_1)

            def _recip1():
                invf1 = scr.tile([4, L], F32, tag="invf", name="invf1")
                nc.vector.reciprocal_approx_fast(invf1[:, :], rms1[:, :])
                _recip1.t = invf1

            def _inv16():
                iv = scr.tile([4, L], F16, tag="invrs", name="invrs1")
                nc.vector.tensor_copy(iv[:, :], _recip1.t[:, :])
                _inv16.t = iv
            fill.append(_recip1)
            fill.append(_inv16)

            # rope pair1 (6 vector ops)
            st1 = {}

            def _rope1(step):
                def f():
                    if step == 0:
                        t = scr.tile([128, L], F16, tag="tC", name="tC1")
                        nc.vector.tensor_mul(t[:, :], E1[:, :], cos_t[:, :])
                        st1["tC"] = t
                    elif step == 1:
                        t = scr.tile([128, L], F16, tag="tD", name="tD1")
                        nc.vector.tensor_mul(t[:, :], O1[:, :], sin_t[:, :])
                        st1["tD"] = t
                    elif step == 2:
                        t = scr.tile([128, L], F16, tag="rA", name="rA1")
                        nc.vector.tensor_sub(t[:, :], st1["tC"][:, :],
                                             st1["tD"][:, :])
                        st1["rA"] = t
                    elif step == 3:
                        t = scr.tile([128, L], F16, tag="tC", name="tC1b")
                        nc.vector.tensor_mul(t[:, :], E1[:, :], sin_t[:, :])
                        st1["tC2"] = t
                    elif step == 4:
                        t = scr.tile([128, L], F16, tag="tD", name="tD1b")
                        nc.vector.tensor_mul(t[:, :], O1[:, :], cos_t[:, :])
                        st1["tD2"] = t
                    else:
                        t = scr.tile([128, L], F16, tag="rB", name="rB1")
                        nc.vector.tensor_add(t[:, :], st1["tC2"][:, :],
                                             st1["tD2"][:, :])
                        st1["rB"] = t
                return f
            for step in range(6):
                fill.append(_rope1(step))

            # gamma*invrms apply for pair1
            def _mk_sX1():
                st1["sE"] = scr.tile([128, L], F16, tag="sE", name="sE1")
                st1["sO"] = scr.tile([128, L], F16, tag="sO", name="sO1")
            fill.append(_mk_sX1)

            def _gapply1(kind, sl):
                def f():
                    rt = st1["rA"] if kind == 0 else st1["rB"]
                    stt = st1["sE"] if kind == 0 else st1["sO"]
                    gsl = gind_t[:, (2 + kind) * 128:(3 + kind) * 128]
                    mm = ap.tile([128, 512], F32, tag="aux", name="m1")
                    nc.tensor.matmul(mm[:, :], gsl,
                                     _inv16.t[:, sl * 512:(sl + 1) * 512],
                                     start=True, stop=True)
                    nc.vector.tensor_mul(
                        stt[:, sl * 512:(sl + 1) * 512],
                        rt[:, sl * 512:(sl + 1) * 512], mm[:, :])
                return f
            for kind in range(2):
                for sl in range(4):
                    fill.append(_gapply1(kind, sl))

            def _reloc1():
                reloc(1, st1["sE"], st1["sO"], engs=[nc.sync, nc.gpsimd])
            fill.append(_reloc1)

            # V1 pass (oc 5) via aux psum, 4 lq chunks x 2 closures each
            v1t = {}

            def _v1mm(lq, half):
                def f():
                    if half == 0:
                        v1t[lq] = ap.tile([128, 512], F32, tag="aux",
                                          name=f"v1_{lq}")
                        qkv_pass_mms({lq: v1t[lq]}, 5, range(0, 4), lqs=(lq,))
                    else:
                        qkv_pass_mms({lq: v1t[lq]}, 5, range(4, 8), lqs=(lq,))
                        nc.vector.tensor_scalar_add(
                            V1c[:, lq * 512:(lq + 1) * 512],
                            v1t[lq][:, :], bias_t[:, 5:6])
                return f
            for lq in range(4):
                fill.append(_v1mm(lq, 0))
                fill.append(_v1mm(lq, 1))

            for lw in range(NKV):
                fill.append(lambda lw=lw: v_transpose(1, V1c, lw))

            # ---- attention blocks ----
            pending = []   # deferred normalize/outproj closures (pop first)

            def _norm_h(p, q0, oo, h):
                def f():
                    # denominator row (partition 64) -> f16 -> broadcast to
                    # partitions 0-63 via matmul, then reciprocal at base 0
                    # (custom-DVE ops mis-handle nonzero base partitions)
                    den16 = dn.tile([65, LQB], F16, tag="den16", name="den16")
                    nc.vector.tensor_copy(den16[64:65, :], oo[64:65, :])
                    dbc = ap.tile([128, 512], F32, tag="aux", name="dbc")
                    nc.tensor.matmul(dbc[0:64, :], ones_t[64:65, :],
                                     den16[64:65, :], start=True, stop=True)
                    rcb = dn.tile([64, LQB], F32, tag="rcb", name="rcb")
                    nc.vector.reciprocal_approx_fast(rcb[:, :], dbc[0:64, :])
                    onrm = dn.tile([64, LQB], F16, tag="onrm", name="onrm")
                    nc.vector.tensor_mul(onrm[:, :], oo[0:64, :], rcb[:, :])
                    nc.sync.dma_start(
                        out=oT[p][h * 64:(h + 1) * 64, q0:q0 + LQB],
                        in_=onrm[:, :])
                return f

            def _av(p, j, oA, oB, e):
                nc.tensor.matmul(oA[:, :], vseq[p][j][:, 0:65],
                                 e[:, 0:LQB],
                                 start=(j == 0), stop=(j == NKV - 1))
                nc.tensor.matmul(oB[:, :], vseq[p][j][:, 65:130],
                                 e[:, LQB:2 * LQB],
                                 start=(j == 0), stop=(j == NKV - 1))

            def outproj_chunk(lw):
                for n in range(2):
                    po = ap.tile([128, 512], F32, tag="aux", name=f"po{lw}_{n}")
                    nc.tensor.matmul(po[:, :], oT[0][:, lw * 128:(lw + 1) * 128],
                                     wo_t[0][:, n * 512:(n + 1) * 512],
                                     start=True, stop=False)
                    nc.tensor.matmul(po[:, :], oT[1][:, lw * 128:(lw + 1) * 128],
                                     wo_t[1][:, n * 512:(n + 1) * 512],
                                     start=False, stop=True)
                    ovh = ov.tile([128, 512], F16, tag="ov", name="ovh")
                    nc.vector.tensor_copy(ovh[:, :], po[:, :])
                    nc.sync.dma_start(
                        out=out_d[lw * 128:(lw + 1) * 128,
                                  n * 512:(n + 1) * 512],
                        in_=ovh[:, :])

            # flat (block, j) stream: no pipeline bubble at block boundaries.
            # AV lags scores/exp by one iteration; each finished block's
            # normalize + out-projection become deferred closures.
            blocks = [(0, qb) for qb in range(4)] + [(1, qb) for qb in range(4)]
            prev = None
            defer = []
            for i in range(8 * NKV):
                bi, j = divmod(i, NKV)
                p, qb = blocks[bi]
                q0 = qb * LQB
                if j == 0:
                    oA = ap.tile([65, LQB], F32, tag="oA", bufs=1, name="oA")
                    oB = ap.tile([65, LQB], F32, tag="oB", bufs=1, name="oB")
                s = ap.tile([128, 2 * LQB], F32, tag="s", bufs=2, name="s")
                nc.tensor.matmul(
                    s[:, 0:LQB],
                    kT[p][0:64, j * 128:(j + 1) * 128],
                    qT[p][0:64, q0:q0 + LQB],
                    start=True, stop=True, tile_position=(0, 0))
                nc.tensor.matmul(
                    s[:, LQB:2 * LQB],
                    kT[p][64:128, j * 128:(j + 1) * 128],
                    qT[p][64:128, q0:q0 + LQB],
                    start=True, stop=True, tile_position=(64, 0))
                e = ep.tile([128, 2 * LQB], F16, tag="e", name="e")
                nc.scalar.activation(e[:, :], s[:, :], AF.Exp)
                if prev is not None:
                    pp, pqb, pj, poA, poB, pe = prev
                    if pj == 0:
                        # defer a block's first AV one extra iteration: its
                        # write into the rotated oA/oB waits the previous
                        # block's normalize-mul, and emitting it now would
                        # stall the in-order PE queue (and the next scores)
                        # behind that vector chain
                        defer.append(prev)
                    else:
                        while defer:
                            dp, dqb, dj, doA, doB, de = defer.pop(0)
                            _av(dp, dj, doA, doB, de)
                        _av(pp, pj, poA, poB, pe)
                        if pj == NKV - 1:
                            pending.append(_norm_h(pp, pqb * LQB, poA, 0))
                            pending.append(_norm_h(pp, pqb * LQB, poB, 1))
                            if pp == 1:
                                pending.extend(
                                    lambda lw=lw: outproj_chunk(lw)
                                    for lw in range(pqb * 4, pqb * 4 + 4))
                prev = (p, qb, j, oA, oB, e)
                # pop deferred work: both normalizes at a block start (so the
                # new block's first AV never precedes the old tiles' readers)
                npop = 2 if j == 0 else 1
                if i >= 40:
                    npop += 1
                for _ in range(npop):
                    if pending:
                        pending.pop(0)()
                    elif fill:
                        fill.pop(0)()
            pp, pqb, pj, poA, poB, pe = prev
            _av(pp, pj, poA, poB, pe)
            pending.append(_norm_h(pp, pqb * LQB, poA, 0))
            pending.append(_norm_h(pp, pqb * LQB, poB, 1))
            pending.extend(lambda lw=lw: outproj_chunk(lw)
                           for lw in range(pqb * 4, pqb * 4 + 4))
            while pending:
                pending.pop(0)()
            while fill:
                fill.pop(0)()

            if dbg:
                dbg_d = {}
                for nm in ("qT0", "qT1", "kT0", "kT1", "oT0", "oT1"):
                    dbg_d[nm] = nc.dram_tensor(f"dbg_{nm}", [128, L], F16,
                                               kind="ExternalOutput")
                for nm in ("rms0", "rms1"):
                    dbg_d[nm] = nc.dram_tensor(f"dbg_{nm}", [4, L], F32,
                                               kind="ExternalOutput")
                for nm in ("iv0", "iv1"):
                    dbg_d[nm] = nc.dram_tensor(f"dbg_{nm}", [4, L], F16,
                                               kind="ExternalOutput")
                for p in range(2):
                    nc.sync.dma_start(out=dbg_d[f"qT{p}"][:, :], in_=qT[p][:, :])
                    nc.sync.dma_start(out=dbg_d[f"kT{p}"][:, :], in_=kT[p][:, :])
                    nc.sync.dma_start(out=dbg_d[f"oT{p}"][:, :], in_=oT[p][:, :])
                nc.sync.dma_start(out=dbg_d["rms0"][:, :], in_=rms0[:, :])
                nc.sync.dma_start(out=dbg_d["rms1"][:, :], in_=rms1[:, :])
                nc.sync.dma_start(out=dbg_d["iv0"][:, :], in_=invrs0[:, :])
                nc.sync.dma_start(out=dbg_d["iv1"][:, :], in_=_inv16.t[:, :])

            ap.release()
            ov.release()
            ep.release()
            dn.release()
            scr.release()
            chk.release()
            lv.release()
            xw.release()
    nc.compile()
    return nc


_PROG = None


def _get_program():
    global _PROG
    if _PROG is None:
        _PROG = _build_program()
    return _PROG


def _core_inputs(core, x, w_qkv, b_qkv, w_out, q_gamma, k_gamma,
                 cos_tab, sin_tab, ind):
    b = core // 4
    hb = (core % 4) * 4
    # row permutation of w_qkv for this core: per pair [E(128) O(128) V(128)]
    perm = []
    for p in range(NPAIR):
        hA, hB = hb + 2 * p, hb + 2 * p + 1
        for h in (hA, hB):                      # E chunk: q evens, k evens
            perm += [h * 64 + 2 * j for j in range(32)]
        for h in (hA, hB):
            perm += [1024 + h * 64 + 2 * j for j in range(32)]
        for h in (hA, hB):                      # O chunk
            perm += [h * 64 + 2 * j + 1 for j in range(32)]
        for h in (hA, hB):
            perm += [1024 + h * 64 + 2 * j + 1 for j in range(32)]
        for h in (hA, hB):                      # V chunk
            perm += [2048 + h * 64 + c for c in range(64)]
    perm = np.asarray(perm)
    w_local = w_qkv[perm]                       # [768, 1024]
    wq = np.ascontiguousarray(w_local.T).astype(np.float16)
    bias6 = np.ascontiguousarray(b_qkv[perm].reshape(6, 128)).astype(np.float32)

    # gamma-scaled indicator lhsT [4, 4*128]: (p, kind) -> [4, 128]
    # 1/sqrt(D) = 1/8 folded into the q-group rows
    gind = np.zeros((4, 4 * 128), np.float32)
    for p in range(NPAIR):
        for kind in range(2):                   # 0=E(evens), 1=O(odds)
            blk = (2 * p + kind) * 128
            for r in range(128):
                g = r // 32
                h = hb + 2 * p + (g % 2)
                ch = 2 * (r % 32) + kind
                gam = q_gamma[h, ch] * 0.125 if g < 2 else k_gamma[h, ch]
                gind[g, blk + r] = gam

    # w_out slice: [256, 1024]
    wo = np.empty((256, 1024), np.float32)
    for p in range(NPAIR):
        for i, h in enumerate((hb + 2 * p, hb + 2 * p + 1)):
            wo[p * 128 + i * 64:p * 128 + (i + 1) * 64, :] = \
                w_out[:, h * 64:(h + 1) * 64].T

    return {
        "xt": np.ascontiguousarray(x[b].T).astype(np.float16),
        "wq": wq,
        "bias6": bias6,
        "cost": cos_tab,
        "sint": sin_tab,
        "ind": ind,
        "gind": gind.astype(np.float16),
        "wo": wo.astype(np.float16),
    }


def kernel(x, w_qkv, b_qkv, w_out, b_out, q_gamma, k_gamma, _trace=False):
    x = np.asarray(x, np.float32)
    w_qkv = np.asarray(w_qkv, np.float32)
    b_qkv = np.asarray(b_qkv, np.float32)
    w_out = np.asarray(w_out, np.float32)
    b_out = np.asarray(b_out, np.float32)
    q_gamma = np.asarray(q_gamma, np.float32)
    k_gamma = np.asarray(k_gamma, np.float32)

    inv_freq = (1.0 / ROPE_THETA ** (np.arange(32, dtype=np.float64) / 32.0))
    ang = np.arange(L, dtype=np.float64)[None, :] * \
        np.tile(inv_freq, 4)[:, None]          # [128, L], row r -> freq r%32
    cos_tab = np.cos(ang).astype(np.float16)
    sin_tab = np.sin(ang).astype(np.float16)
    ind = np.zeros((128, 4), np.float16)
    for r in range(128):
        ind[r, r // 32] = 1.0

    nc = _get_program()
    in_maps = [_core_inputs(c, x, w_qkv, b_qkv, w_out, q_gamma, k_gamma,
                            cos_tab, sin_tab, ind)
               for c in range(NCORES)]
    r = run_bass_kernel_spmd(nc, in_maps, list(range(NCORES)), trace=_trace)
    out = np.zeros((B, L, C), np.float32)
    for c in range(NCORES):
        out[c // 4] += r.results[c]["out"].astype(np.float32)
    out += b_out[None, None, :]
    if _trace:
        kernel._last_results = r
    return out


# revision 68
# speedup vs baseline: 1.0300x; 1.0300x over previous
"""MultiHeadAttention (RoPE + QK-RMSNorm, non-causal) on 8 trn2 NeuronCores.

Sharding: batch (2) x head-groups (4 heads each) -> 8 cores. All-f16 dataflow
(inputs converted host-side; fp32 PSUM accumulation). Per core:
  - streamed QKV passes (weight-stationary, cc-accumulated): E0 O0 | E1 | O1 | V0
    pre-attention, V1 interleaved into attention as filler work
  - RMS inverse via scalar Sqrt + vector fast reciprocal (no act-table thrash)
  - RoPE on vector in f16; gamma*invrms broadcast via small PE matmuls
  - attention in (pair, 512-query-block) blocks: per kv-chunk j one
    [128,1024] exp on the scalar engine (the pace-setter), 2 score MMs +
    2 AV MMs (N=512, f16); denominator via ones-column of V
  - pair-1 prep, V1 projection+transpose and the first output-projection
    chunks run as fillers inside the attention instruction streams
Host: sums the 4 partials per batch (f16 -> f32) and adds b_out.
"""
import math
import numpy as np

import concourse.bass as bass
from concourse import bacc
import concourse.mybir as mybir
import concourse.tile as tile
from concourse.bass_utils import run_bass_kernel_spmd
from concourse.masks import make_identity

F32 = mybir.dt.float32
F16 = mybir.dt.float16
AF = mybir.ActivationFunctionType

B, L, C, H, D = 2, 2048, 1024, 16, 64
NCORES = 8
ROPE_THETA = 10000.0
RMS_EPS = 1e-6
NPAIR = 2
LQB = 512        # query block size in attention
NKV = L // 128   # 16 kv chunks


def _build_program(dbg=False):
    nc = bacc.Bacc("TRN2", target_bir_lowering=False, debug=False)

    xt_d = nc.dram_tensor("xt", [C, L], F16, kind="ExternalInput")
    wq_d = nc.dram_tensor("wq", [C, 768], F16, kind="ExternalInput")
    bias_d = nc.dram_tensor("bias6", [6, 128], F32, kind="ExternalInput")
    cos_d = nc.dram_tensor("cost", [32, L], F16, kind="ExternalInput")
    sin_d = nc.dram_tensor("sint", [32, L], F16, kind="ExternalInput")
    ind_d = nc.dram_tensor("ind", [128, 4], F16, kind="ExternalInput")
    gind_d = nc.dram_tensor("gind", [4, 4 * 128], F16, kind="ExternalInput")
    wo_d = nc.dram_tensor("wo", [256, 1024], F16, kind="ExternalInput")
    out_d = nc.dram_tensor("out", [L, C], F16, kind="ExternalOutput")

    with tile.TileContext(nc) as tc:
        with tc.tile_pool(name="const", bufs=1) as cp:
            # ---- input DMAs: wq/xt stream first (pass1 is gated on them) ----
            # spread input loads across engine DMA queues for parallel HBM
            # streams (each engine trigger feeds its own queue)
            xw = tc.alloc_tile_pool(name="xw", bufs=1)
            dmae = [nc.sync, nc.scalar, nc.gpsimd]
            xt_sb, wq_sb = [], []
            for cc in range(8):
                wqi = xw.tile([128, 768], F16, tag=f"w{cc}", name=f"w{cc}")
                dmae[cc % 3].dma_start(out=wqi,
                                       in_=wq_d[cc * 128:(cc + 1) * 128, :])
                wq_sb.append(wqi)
                xti = xw.tile([128, L], F16, tag=f"x{cc}", name=f"x{cc}")
                dmae[(cc + 1) % 3].dma_start(
                    out=xti, in_=xt_d[cc * 128:(cc + 1) * 128, :])
                xt_sb.append(xti)
            bias_t = cp.tile([128, 6], F32, tag="bias")
            nc.sync.dma_start(out=bias_t, in_=bias_d[:, :].transpose([1, 0]))
            # rope tables repeat every 32 rows: DMA [32,L] and double on-chip
            # (saves 768KB of input-stream HBM traffic)
            cos_t = cp.tile([128, L], F16, tag="cos")
            nc.scalar.dma_start(out=cos_t[0:32, :], in_=cos_d[:, :])
            nc.scalar.dma_start(out=cos_t[32:64, :], in_=cos_t[0:32, :])
            nc.scalar.dma_start(out=cos_t[64:128, :], in_=cos_t[0:64, :])
            sin_t = cp.tile([128, L], F16, tag="sin")
            nc.gpsimd.dma_start(out=sin_t[0:32, :], in_=sin_d[:, :])
            nc.gpsimd.dma_start(out=sin_t[32:64, :], in_=sin_t[0:32, :])
            nc.gpsimd.dma_start(out=sin_t[64:128, :], in_=sin_t[0:64, :])
            ind_t = cp.tile([128, 4], F16, tag="ind")
            nc.sync.dma_start(out=ind_t, in_=ind_d[:, :])
            gind_t = cp.tile([4, 4 * 128], F16, tag="gind")
            nc.sync.dma_start(out=gind_t, in_=gind_d[:, :])
            wo_t = [cp.tile([128, 1024], F16, tag=f"wo{p}", name=f"wo{p}")
                    for p in range(2)]
            for p in range(2):
                nc.sync.dma_start(out=wo_t[p],
                                  in_=wo_d[p * 128:(p + 1) * 128, :])
            eps_t = cp.tile([4, 1], F32, tag="eps")
            nc.vector.memset(eps_t[:, :], RMS_EPS)
            # pre-warm the Identity and Sqrt act-table sets while everything
            # waits on input DMA — their first real uses sit on the
            # pre-attention critical cascade (bias-adds gate the QKV passes)
            warm_t = cp.tile([4, 1], F32, tag="warm")
            nc.scalar.add(warm_t[:, :], eps_t[:, :], 0.0)
            nc.scalar.activation(warm_t[:, :], eps_t[:, :], AF.Sqrt)
            ones_t = cp.tile([128, 64], F16, tag="ones")
            nc.vector.memset(ones_t[:, :], 1.0)
            onecol = cp.tile([128, 2], F16, tag="onecol")
            nc.vector.memset(onecol[:, :], 1.0)
            ident = cp.tile([128, 128], F16, tag="ident")
            make_identity(nc, ident[:, :])

            # ---- long-lived attention operands ----
            lv = tc.alloc_tile_pool(name="live", bufs=1)
            qT, kT, vseq = [], [], []
            for p in range(NPAIR):
                qT.append(lv.tile([128, L], F16, tag=f"qT{p}", name=f"qT{p}"))
                kT.append(lv.tile([128, L], F16, tag=f"kT{p}", name=f"kT{p}"))
                vseq.append([lv.tile([128, 130], F16, tag=f"vs{p}_{lw}",
                                     name=f"vs{p}_{lw}") for lw in range(NKV)])
            oT = [lv.tile([128, L], F16, tag=f"oT{p}", name=f"oT{p}")
                  for p in range(NPAIR)]

            chk = tc.alloc_tile_pool(name="chunks", bufs=1)
            # oc order in wq columns: E0 O0 V0 E1 O1 V1
            chunks = [chk.tile([128, L], F16, tag=f"c{i}", name=f"c{i}")
                      for i in range(6)]
            E0, O0, V0c, E1, O1, V1c = (chunks[0], chunks[1], chunks[2],
                                        chunks[3], chunks[4], chunks[5])

            scr = tc.alloc_tile_pool(name="scratch", bufs=1)
            # rope temporaries / squares (rotating tags)
            # rms/invf fp32, invrs f16 per pair (tags rotate p0 -> p1)
            dn = tc.alloc_tile_pool(name="dn", bufs=2)

            def qkv_pass_mms(ps_tiles, oc, cc_range, lqs=(0, 1, 2, 3)):
                for cc in cc_range:
                    for lq in lqs:
                        nc.tensor.matmul(
                            ps_tiles[lq][:, :],
                            wq_sb[cc][:, oc * 128:(oc + 1) * 128],
                            xt_sb[cc][:, lq * 512:(lq + 1) * 512],
                            start=(cc == 0), stop=(cc == 7))

            def bias_add(ps_tiles, oc, lq):
                # scalar engine (idle pre-attention); vector is the critical
                # pre-attention chain
                nc.scalar.add(
                    chunks[oc][:, lq * 512:(lq + 1) * 512],
                    ps_tiles[lq][:, :], bias_t[:, oc:oc + 1])

            # ---- HAM warm-up: ~3us of dense dummy matmuls while the input
            # DMA streams, so the PE clock gate is at 2.4GHz before the real
            # QKV passes (traces show it otherwise stays at 1.2GHz to ~18us)
            wu = tc.alloc_tile_pool(name="wu", bufs=1, space="PSUM")
            wt = wu.tile([128, 512], F32, tag="wu", name="wt")
            for _ in range(30):
                nc.tensor.matmul(wt[:, 0:128], ident[:, :], ident[:, :],
                                 start=True, stop=True)
            wu.release()

            # ---- pass 1: E0 + O0 (8 psum banks, dma-gated) ----
            q1 = tc.alloc_tile_pool(name="q1", bufs=1, space="PSUM")
            ps1 = {(oc, lq): q1.tile([128, 512], F32, tag=f"p{oc}_{lq}",
                                     name=f"p{oc}_{lq}")
                   for oc in (0, 1) for lq in range(4)}
            for cc in range(8):
                for oc in (0, 1):
                    for lq in range(4):
                        nc.tensor.matmul(
                            ps1[(oc, lq)][:, :],
                            wq_sb[cc][:, oc * 128:(oc + 1) * 128],
                            xt_sb[cc][:, lq * 512:(lq + 1) * 512],
                            start=(cc == 0), stop=(cc == 7))
            for oc in (0, 1):
                for lq in range(4):
                    nc.vector.tensor_scalar_add(
                        chunks[oc][:, lq * 512:(lq + 1) * 512],
                        ps1[(oc, lq)][:, :], bias_t[:, oc:oc + 1])
            q1.release()

            # ---- passes 2-4 (4 banks) + ps4_0 (4 banks) ----
            q2 = tc.alloc_tile_pool(name="q2", bufs=1, space="PSUM")
            p4p = tc.alloc_tile_pool(name="p4p", bufs=1, space="PSUM")
            ps4_0 = p4p.tile([4, L], F32, tag="ps4", name="ps4_0")

            # vector: squares of pair0 (reads E0/O0 after pass-1 bias adds)
            sqE = scr.tile([128, L], F16, tag="tC", name="sqE0")
            nc.vector.tensor_mul(sqE[:, :], E0[:, :], E0[:, :])
            sqO = scr.tile([128, L], F16, tag="tD", name="sqO0")
            nc.vector.tensor_mul(sqO[:, :], O0[:, :], O0[:, :])

            # PE: pass2 (E1) cc 0..4, then ps4_0 MMs (sq ready by then), rest
            t2 = [q2.tile([128, 512], F32, tag=f"t{lq}", name=f"e1_{lq}")
                  for lq in range(4)]
            qkv_pass_mms(t2, 3, range(0, 5))
            for sl in range(4):
                nc.tensor.matmul(ps4_0[:, sl * 512:(sl + 1) * 512],
                                 ind_t[:, :],
                                 sqE[:, sl * 512:(sl + 1) * 512],
                                 start=True, stop=False)
                nc.tensor.matmul(ps4_0[:, sl * 512:(sl + 1) * 512],
                                 ind_t[:, :],
                                 sqO[:, sl * 512:(sl + 1) * 512],
                                 start=False, stop=True)
            qkv_pass_mms(t2, 3, range(5, 8))
            for lq in range(4):
                bias_add(t2, 3, lq)
            # scalar: rms0 = sqrt(ps4_0/64 + eps) — after the E1 bias-adds:
            # those gate pass3's psum-tag rotation, Sqrt0 only feeds recip0
            # which waits on the longer vector rope chain anyway
            rms0 = scr.tile([4, L], F32, tag="rms", name="rms0")
            nc.scalar.activation(rms0[:, :], ps4_0[:, :], AF.Sqrt,
                                 scale=1.0 / 64.0, bias=eps_t[:, 0:1])

            # vector: rope pair0 (f16)
            tC = scr.tile([128, L], F16, tag="tC", name="tC0")
            nc.vector.tensor_mul(tC[:, :], E0[:, :], cos_t[:, :])
            tD = scr.tile([128, L], F16, tag="tD", name="tD0")
            nc.vector.tensor_mul(tD[:, :], O0[:, :], sin_t[:, :])
            rA = scr.tile([128, L], F16, tag="rA", name="rA0")
            nc.vector.tensor_sub(rA[:, :], tC[:, :], tD[:, :])
            tC2 = scr.tile([128, L], F16, tag="tC", name="tC0b")
            nc.vector.tensor_mul(tC2[:, :], E0[:, :], sin_t[:, :])
            tD2 = scr.tile([128, L], F16, tag="tD", name="tD0b")
            nc.vector.tensor_mul(tD2[:, :], O0[:, :], cos_t[:, :])
            rB = scr.tile([128, L], F16, tag="rB", name="rB0")
            nc.vector.tensor_add(rB[:, :], tC2[:, :], tD2[:, :])

            # vector: invrs0 = recip(rms0) -> f16
            invf0 = scr.tile([4, L], F32, tag="invf", name="invf0")
            nc.vector.reciprocal_approx_fast(invf0[:, :], rms0[:, :])
            invrs0 = scr.tile([4, L], F16, tag="invrs", name="invrs0")
            nc.vector.tensor_copy(invrs0[:, :], invf0[:, :])

            # PE: pass3 (O1) so the whole pair-1 rms chain can run
            # pre-attention (keeps Sqrt out of the attention exp stream)
            t3 = [q2.tile([128, 512], F32, tag=f"t{lq}", name=f"o1_{lq}")
                  for lq in range(4)]
            qkv_pass_mms(t3, 4, range(8))
            for lq in range(4):
                bias_add(t3, 4, lq)

            # pair1 squares + ps4_1 + Sqrt1 (pre-attention)
            sqE1 = scr.tile([128, L], F16, tag="tC", name="sqE1")
            nc.vector.tensor_mul(sqE1[:, :], E1[:, :], E1[:, :])
            sqO1 = scr.tile([128, L], F16, tag="tD", name="sqO1")
            nc.vector.tensor_mul(sqO1[:, :], O1[:, :], O1[:, :])
            ps4_1 = p4p.tile([4, L], F32, tag="ps4", name="ps4_1")
            for sl in range(4):
                nc.tensor.matmul(ps4_1[:, sl * 512:(sl + 1) * 512],
                                 ind_t[:, :],
                                 sqE1[:, sl * 512:(sl + 1) * 512],
                                 start=True, stop=False)
                nc.tensor.matmul(ps4_1[:, sl * 512:(sl + 1) * 512],
                                 ind_t[:, :],
                                 sqO1[:, sl * 512:(sl + 1) * 512],
                                 start=False, stop=True)
            rms1 = scr.tile([4, L], F32, tag="rms", name="rms1")
            nc.scalar.activation(rms1[:, :], ps4_1[:, :], AF.Sqrt,
                                 scale=1.0 / 64.0, bias=eps_t[:, 0:1])
            p4p.release()

            # gamma*invrms broadcast + apply for pair0 -> sE/sO, then reloc
            aux0 = tc.alloc_tile_pool(name="aux0", bufs=2, space="PSUM")
            sE = scr.tile([128, L], F16, tag="sE", name="sE0")
            sO = scr.tile([128, L], F16, tag="sO", name="sO0")
            for kind, (rt, st) in enumerate(((rA, sE), (rB, sO))):
                gsl = gind_t[:, kind * 128:(kind + 1) * 128]
                for sl in range(4):
                    mm = aux0.tile([128, 512], F32, tag="aux0", name="m0")
                    nc.tensor.matmul(mm[:, :], gsl,
                                     invrs0[:, sl * 512:(sl + 1) * 512],
                                     start=True, stop=True)
                    nc.vector.tensor_mul(
                        st[:, sl * 512:(sl + 1) * 512],
                        rt[:, sl * 512:(sl + 1) * 512], mm[:, :])

            def reloc(p, sEt, sOt, engs=dmae):
                # spread across engine DMA queues: the 8 copies gate the
                # first score matmuls. Mid-attention callers must exclude
                # the scalar engine: a trigger waiting on the pair-1 apply
                # would block the whole exp stream behind it.
                n = [0]

                def _d(out, in_):
                    engs[n[0] % len(engs)].dma_start(out=out, in_=in_)
                    n[0] += 1
                for blk in range(2):
                    _d(qT[p][blk * 64:blk * 64 + 32, :],
                       sEt[blk * 32:(blk + 1) * 32, :])
                    _d(qT[p][blk * 64 + 32:blk * 64 + 64, :],
                       sOt[blk * 32:(blk + 1) * 32, :])
                    _d(kT[p][blk * 64:blk * 64 + 32, :],
                       sEt[64 + blk * 32:64 + (blk + 1) * 32, :])
                    _d(kT[p][blk * 64 + 32:blk * 64 + 64, :],
                       sOt[64 + blk * 32:64 + (blk + 1) * 32, :])

            reloc(0, sE, sO)

            # PE: pass4 (V0), bias on scalar
            t4 = [q2.tile([128, 512], F32, tag=f"t{lq}", name=f"v0_{lq}")
                  for lq in range(4)]
            qkv_pass_mms(t4, 2, range(8))
            for lq in range(4):
                bias_add(t4, 2, lq)

            aux0.release()
            q2.release()

            # ================= attention phase =================
            # pool default bufs=2 (tags "s", "aux"); oA/oB override to 1.
            # PSUM budget: s 2x4KB + oA 2KB + oB 2KB + aux 2x2KB = 16KB = 8 banks
            ap = tc.alloc_tile_pool(name="att", bufs=2, space="PSUM")
            # deep e rotation: lets the exp stream run ahead while early AVs
            # wait for the V0 transpose fillers to land
            ep = tc.alloc_tile_pool(name="exp", bufs=8)
            ov = tc.alloc_tile_pool(name="ov", bufs=2)

            def v_transpose(p, Vc, lw):
                pt = ap.tile([128, 128], F16, tag="aux", name=f"pt{p}_{lw}")
                nc.tensor.transpose(pt[:, :], Vc[:, lw * 128:(lw + 1) * 128],
                                    ident[:, :])
                vv = vseq[p][lw].rearrange("a (h x) -> a h x", h=2)
                nc.vector.tensor_copy(
                    vv[:, :, 0:64],
                    pt[:, :].rearrange("a (h x) -> a h x", h=2))
                nc.vector.tensor_copy(vv[:, :, 64], onecol[:, :])

            # ---- filler closures, popped inside the attention p0 loop ----
            fill = []

            # V0 transposes as the FIRST fillers: vtr_j pops at iteration j,
            # one ahead of AV_j (emitted at iteration j+1), so they interleave
            # with the exp stream instead of gating attention start
            for lw in range(NKV):
                fill.append(lambda lw=lw: v_transpose(0, V0c, lw))

            def _recip1():
                invf1 = scr.tile([4, L], F32, tag="invf", name="invf1")
                nc.vector.reciprocal_approx_fast(invf1[:, :], rms1[:, :])
                _recip1.t = invf1

            def _inv16():
                iv = scr.tile([4, L], F16, tag="invrs", name="invrs1")
                nc.vector.tensor_copy(iv[:, :], _recip1.t[:, :])
                _inv16.t = iv
            fill.append(_recip1)
            fill.append(_inv16)

            # rope pair1 (6 vector ops)
            st1 = {}

            def _rope1(step):
                def f():
                    if step == 0:
                        t = scr.tile([128, L], F16, tag="tC", name="tC1")
                        nc.vector.tensor_mul(t[:, :], E1[:, :], cos_t[:, :])
                        st1["tC"] = t
                    elif step == 1:
                        t = scr.tile([128, L], F16, tag="tD", name="tD1")
                        nc.vector.tensor_mul(t[:, :], O1[:, :], sin_t[:, :])
                        st1["tD"] = t
                    elif step == 2:
                        t = scr.tile([128, L], F16, tag="rA", name="rA1")
                        nc.vector.tensor_sub(t[:, :], st1["tC"][:, :],
                                             st1["tD"][:, :])
                        st1["rA"] = t
                    elif step == 3:
                        t = scr.tile([128, L], F16, tag="tC", name="tC1b")
                        nc.vector.tensor_mul(t[:, :], E1[:, :], sin_t[:, :])
                        st1["tC2"] = t
                    elif step == 4:
                        t = scr.tile([128, L], F16, tag="tD", name="tD1b")
                        nc.vector.tensor_mul(t[:, :], O1[:, :], cos_t[:, :])
                        st1["tD2"] = t
                    else:
                        t = scr.tile([128, L], F16, tag="rB", name="rB1")
                        nc.vector.tensor_add(t[:, :], st1["tC2"][:, :],
                                             st1["tD2"][:, :])
                        st1["rB"] = t
                return f
            for step in range(6):
                fill.append(_rope1(step))

            # gamma*invrms apply for pair1
            def _mk_sX1():
                st1["sE"] = scr.tile([128, L], F16, tag="sE", name="sE1")
                st1["sO"] = scr.tile([128, L], F16, tag="sO", name="sO1")
            fill.append(_mk_sX1)

            def _gapply1(kind, sl):
                def f():
                    rt = st1["rA"] if kind == 0 else st1["rB"]
                    stt = st1["sE"] if kind == 0 else st1["sO"]
                    gsl = gind_t[:, (2 + kind) * 128:(3 + kind) * 128]
                    mm = ap.tile([128, 512], F32, tag="aux", name="m1")
                    nc.tensor.matmul(mm[:, :], gsl,
                                     _inv16.t[:, sl * 512:(sl + 1) * 512],
                                     start=True, stop=True)
                    nc.vector.tensor_mul(
                        stt[:, sl * 512:(sl + 1) * 512],
                        rt[:, sl * 512:(sl + 1) * 512], mm[:, :])
                return f
            for kind in range(2):
                for sl in range(4):
                    fill.append(_gapply1(kind, sl))

            def _reloc1():
                reloc(1, st1["sE"], st1["sO"], engs=[nc.sync, nc.gpsimd])
            fill.append(_reloc1)

            # V1 pass (oc 5) via aux psum, 4 lq chunks x 2 closures each
            v1t = {}

            def _v1mm(lq, half):
                def f():
                    if half == 0:
                        v1t[lq] = ap.tile([128, 512], F32, tag="aux",
                                          name=f"v1_{lq}")
                        qkv_pass_mms({lq: v1t[lq]}, 5, range(0, 4), lqs=(lq,))
                    else:
                        qkv_pass_mms({lq: v1t[lq]}, 5, range(4, 8), lqs=(lq,))
                        nc.vector.tensor_scalar_add(
                            V1c[:, lq * 512:(lq + 1) * 512],
                            v1t[lq][:, :], bias_t[:, 5:6])
                return f
            for lq in range(4):
                fill.append(_v1mm(lq, 0))
                fill.append(_v1mm(lq, 1))

            for lw in range(NKV):
                fill.append(lambda lw=lw: v_transpose(1, V1c, lw))

            # ---- attention blocks ----
            pending = []   # deferred normalize/outproj closures (pop first)

            def _norm_h(p, q0, oo, h):
                def f():
                    # denominator row (partition 64) -> f16 -> broadcast to
                    # partitions 0-63 via matmul, then reciprocal at base 0
                    # (custom-DVE ops mis-handle nonzero base partitions)
                    den16 = dn.tile([65, LQB], F16, tag="den16", name="den16")
                    nc.vector.tensor_copy(den16[64:65, :], oo[64:65, :])
                    dbc = ap.tile([128, 512], F32, tag="aux", name="dbc")
                    nc.tensor.matmul(dbc[0:64, :], ones_t[64:65, :],
                                     den16[64:65, :], start=True, stop=True)
                    rcb = dn.tile([64, LQB], F32, tag="rcb", name="rcb")
                    nc.vector.reciprocal_approx_fast(rcb[:, :], dbc[0:64, :])
                    onrm = dn.tile([64, LQB], F16, tag="onrm", name="onrm")
                    nc.vector.tensor_mul(onrm[:, :], oo[0:64, :], rcb[:, :])
                    nc.sync.dma_start(
                        out=oT[p][h * 64:(h + 1) * 64, q0:q0 + LQB],
                        in_=onrm[:, :])
                return f

            def _av(p, j, oA, oB, e):
                nc.tensor.matmul(oA[:, :], vseq[p][j][:, 0:65],
                                 e[:, 0:LQB],
                                 start=(j == 0), stop=(j == NKV - 1))
                nc.tensor.matmul(oB[:, :], vseq[p][j][:, 65:130],
                                 e[:, LQB:2 * LQB],
                                 start=(j == 0), stop=(j == NKV - 1))

            def outproj_chunk(lw):
                for n in range(2):
                    po = ap.tile([128, 512], F32, tag="aux", name=f"po{lw}_{n}")
                    nc.tensor.matmul(po[:, :], oT[0][:, lw * 128:(lw + 1) * 128],
                                     wo_t[0][:, n * 512:(n + 1) * 512],
                                     start=True, stop=False)
                    nc.tensor.matmul(po[:, :], oT[1][:, lw * 128:(lw + 1) * 128],
                                     wo_t[1][:, n * 512:(n + 1) * 512],
                                     start=False, stop=True)
                    ovh = ov.tile([128, 512], F16, tag="ov", name="ovh")
                    nc.vector.tensor_copy(ovh[:, :], po[:, :])
                    nc.sync.dma_start(
                        out=out_d[lw * 128:(lw + 1) * 128,
                                  n * 512:(n + 1) * 512],
                        in_=ovh[:, :])

            # flat (block, j) stream: no pipeline bubble at block boundaries.
            # AV lags scores/exp by one iteration; each finished block's
            # normalize + out-projection become deferred closures.
            blocks = [(0, qb) for qb in range(4)] + [(1, qb) for qb in range(4)]
            prev = None
            defer = []
            for i in range(8 * NKV):
                bi, j = divmod(i, NKV)
                p, qb = blocks[bi]
                q0 = qb * LQB
                if j == 0:
                    oA = ap.tile([65, LQB], F32, tag="oA", bufs=1, name="oA")
                    oB = ap.tile([65, LQB], F32, tag="oB", bufs=1, name="oB")
                s = ap.tile([128, 2 * LQB], F32, tag="s", bufs=2, name="s")
                nc.tensor.matmul(
                    s[:, 0:LQB],
                    kT[p][0:64, j * 128:(j + 1) * 128],
                    qT[p][0:64, q0:q0 + LQB],
                    start=True, stop=True, tile_position=(0, 0))
                nc.tensor.matmul(
                    s[:, LQB:2 * LQB],
                    kT[p][64:128, j * 128:(j + 1) * 128],
                    qT[p][64:128, q0:q0 + LQB],
                    start=True, stop=True, tile_position=(64, 0))
                e = ep.tile([128, 2 * LQB], F16, tag="e", name="e")
                nc.scalar.activation(e[:, :], s[:, :], AF.Exp)
                if prev is not None:
                    pp, pqb, pj, poA, poB, pe = prev
                    if pj == 0:
                        # defer a block's first AV one extra iteration: its
                        # write into the rotated oA/oB waits the previous
                        # block's normalize-mul, and emitting it now would
                        # stall the in-order PE queue (and the next scores)
                        # behind that vector chain
                        defer.append(prev)
                    else:
                        while defer:
                            dp, dqb, dj, doA, doB, de = defer.pop(0)
                            _av(dp, dj, doA, doB, de)
                        _av(pp, pj, poA, poB, pe)
                        if pj == NKV - 1:
                            pending.append(_norm_h(pp, pqb * LQB, poA, 0))
                            pending.append(_norm_h(pp, pqb * LQB, poB, 1))
                            if pp == 1:
                                pending.extend(
                                    lambda lw=lw: outproj_chunk(lw)
                                    for lw in range(pqb * 4, pqb * 4 + 4))
                prev = (p, qb, j, oA, oB, e)
                # pop deferred work: both normalizes at a block start (so the
                # new block's first AV never precedes the old tiles' readers)
                npop = 2 if j == 0 else 1
                if i >= 40:
                    npop += 1
                for _ in range(npop):
                    if pending:
                        pending.pop(0)()
                    elif fill:
                        fill.pop(0)()
            pp, pqb, pj, poA, poB, pe = prev
            _av(pp, pj, poA, poB, pe)
            pending.append(_norm_h(pp, pqb * LQB, poA, 0))
            pending.append(_norm_h(pp, pqb * LQB, poB, 1))
            pending.extend(lambda lw=lw: outproj_chunk(lw)
                           for lw in range(pqb * 4, pqb * 4 + 4))
            while pending:
                pending.pop(0)()
            while fill:
                fill.pop(0)()

            if dbg:
                dbg_d = {}
                for nm in ("qT0", "qT1", "kT0", "kT1", "oT0", "oT1"):
                    dbg_d[nm] = nc.dram_tensor(f"dbg_{nm}", [128, L], F16,
                                               kind="ExternalOutput")
                for nm in ("rms0", "rms1"):
                    dbg_d[nm] = nc.dram_tensor(f"dbg_{nm}", [4, L], F32,
                                               kind="ExternalOutput")
                for nm in ("iv0", "iv1"):
                    dbg_d[nm] = nc.dram_tensor(f"dbg_{nm}", [4, L], F16,
                                               kind="ExternalOutput")
                for p in range(2):
                    nc.sync.dma_start(out=dbg_d[f"qT{p}"][:, :], in_=qT[p][:, :])
                    nc.sync.dma_start(out=dbg_d[f"kT{p}"][:, :], in_=kT[p][:, :])
                    nc.sync.dma_start(out=dbg_d[f"oT{p}"][:, :], in_=oT[p][:, :])
                nc.sync.dma_start(out=dbg_d["rms0"][:, :], in_=rms0[:, :])
                nc.sync.dma_start(out=dbg_d["rms1"][:, :], in_=rms1[:, :])
                nc.sync.dma_start(out=dbg_d["iv0"][:, :], in_=invrs0[:, :])
                nc.sync.dma_start(out=dbg_d["iv1"][:, :], in_=_inv16.t[:, :])

            ap.release()
            ov.release()
            ep.release()
            dn.release()
            scr.release()
            chk.release()
            lv.release()
            xw.release()
    nc.compile()
    return nc


_PROG = None


def _get_program():
    global _PROG
    if _PROG is None:
        _PROG = _build_program()
    return _PROG


def _core_inputs(core, x, w_qkv, b_qkv, w_out, q_gamma, k_gamma,
                 cos_tab, sin_tab, ind):
    b = core // 4
    hb = (core % 4) * 4
    # row permutation of w_qkv for this core: per pair [E(128) O(128) V(128)]
    perm = []
    for p in range(NPAIR):
        hA, hB = hb + 2 * p, hb + 2 * p + 1
        for h in (hA, hB):                      # E chunk: q evens, k evens
            perm += [h * 64 + 2 * j for j in range(32)]
        for h in (hA, hB):
            perm += [1024 + h * 64 + 2 * j for j in range(32)]
        for h in (hA, hB):                      # O chunk
            perm += [h * 64 + 2 * j + 1 for j in range(32)]
        for h in (hA, hB):
            perm += [1024 + h * 64 + 2 * j + 1 for j in range(32)]
        for h in (hA, hB):                      # V chunk
            perm += [2048 + h * 64 + c for c in range(64)]
    perm = np.asarray(perm)
    w_local = w_qkv[perm]                       # [768, 1024]
    wq = np.ascontiguousarray(w_local.T).astype(np.float16)
    bias6 = np.ascontiguousarray(b_qkv[perm].reshape(6, 128)).astype(np.float32)

    # gamma-scaled indicator lhsT [4, 4*128]: (p, kind) -> [4, 128]
    # 1/sqrt(D) = 1/8 folded into the q-group rows
    gind = np.zeros((4, 4 * 128), np.float32)
    for p in range(NPAIR):
        for kind in range(2):                   # 0=E(evens), 1=O(odds)
            blk = (2 * p + kind) * 128
            for r in range(128):
                g = r // 32
                h = hb + 2 * p + (g % 2)
                ch = 2 * (r % 32) + kind
                gam = q_gamma[h, ch] * 0.125 if g < 2 else k_gamma[h, ch]
                gind[g, blk + r] = gam

    # w_out slice: [256, 1024]
    wo = np.empty((256, 1024), np.float32)
    for p in range(NPAIR):
        for i, h in enumerate((hb + 2 * p, hb + 2 * p + 1)):
            wo[p * 128 + i * 64:p * 128 + (i + 1) * 64, :] = \
                w_out[:, h * 64:(h + 1) * 64].T

    return {
        "xt": np.ascontiguousarray(x[b].T).astype(np.float16),
        "wq": wq,
        "bias6": bias6,
        "cost": cos_tab,
        "sint": sin_tab,
        "ind": ind,
        "gind": gind.astype(np.float16),
        "wo": wo.astype(np.float16),
    }


def kernel(x, w_qkv, b_qkv, w_out, b_out, q_gamma, k_gamma, _trace=False):
    x = np.asarray(x, np.float32)
    w_qkv = np.asarray(w_qkv, np.float32)
    b_qkv = np.asarray(b_qkv, np.float32)
    w_out = np.asarray(w_out, np.float32)
    b_out = np.asarray(b_out, np.float32)
    q_gamma = np.asarray(q_gamma, np.float32)
    k_gamma = np.asarray(k_gamma, np.float32)

    inv_freq = (1.0 / ROPE_THETA ** (np.arange(32, dtype=np.float64) / 32.0))
    ang = np.arange(L, dtype=np.float64)[None, :] * \
        inv_freq[:, None]                      # [32, L]; replicated on-chip
    cos_tab = np.cos(ang).astype(np.float16)
    sin_tab = np.sin(ang).astype(np.float16)
    ind = np.zeros((128, 4), np.float16)
    for r in range(128):
        ind[r, r // 32] = 1.0

    nc = _get_program()
    in_maps = [_core_inputs(c, x, w_qkv, b_qkv, w_out, q_gamma, k_gamma,
                            cos_tab, sin_tab, ind)
               for c in range(NCORES)]
    r = run_bass_kernel_spmd(nc, in_maps, list(range(NCORES)), trace=_trace)
    out = np.zeros((B, L, C), np.float32)
    for c in range(NCORES):
        out[c // 4] += r.results[c]["out"].astype(np.float32)
    out += b_out[None, None, :]
    if _trace:
        kernel._last_results = r
    return out
